# revision 32
# baseline (speedup 1.0000x reference)
import numpy as np

# nn_MultiHeadedAttention: B=4, S=2048, D_MODEL=1024, H=16, D_K=64, fp32.
# Sharding: 8 cores = 4 batches x 2 head-groups (8 heads each).
# Per-core pipeline (PE-bound, matmuls fp32r at 1 cyc/row):
#   V-proj -> K-proj -> per qc: [Q-proj(qc) -> attention(qc, p=0..3)] -> out-proj
# Attention is software-pipelined one iteration deep: iteration j emits
# scores(j) then PV+normalize of j-1, so the exp of scores(j) has a full
# iteration of slack and the PE never waits on the activation engines.
# Scores ST[k,q] as row-tiled head pairs (fp32r, concurrent on HW), PV in
# bf16 with appended ones-column (softmax denominator free in row 64), exp
# split: exact on ACT (11/16 chunks), Schraudolph bit-trick int16->bf16 on
# DVE (5/16). Fast-approx reciprocal (after a partition-0 staging copy),
# normalize via gpsimd partition_broadcast + DVE multiply written into QT's
# consumed columns (attention output aliases QT). Host sums the two
# head-group partials and adds bo.

B, S, D, H, DK = 4, 2048, 1024, 16, 64
NCORES = 8
DG = 512  # dims per head-group (8 heads x 64)

# Schraudolph exp -> bf16 bits: int16(st*A2 + B2) ~= bf16(exp(st/8))
EXP_A2 = float(2**7 * 0.125 / np.log(2.0))
EXP_B2 = float(127 * 2**7 - 290000.0 / 65536.0)
DVE_KBS = (3, 7, 9, 11, 13, 15)  # kb chunks exp'd on Vector via bit-trick

_NC_CACHE = {}
LAST_EXEC_NS = None


def _build_nc():
    import concourse.bacc as bacc
    import concourse.bass as bass
    import concourse.tile as tile
    from concourse import mybir

    F32R = mybir.dt.float32r
    F32 = mybir.dt.float32
    BF16 = mybir.dt.bfloat16
    I16 = mybir.dt.int16
    EXP = mybir.ActivationFunctionType.Exp
    MULT = mybir.AluOpType.mult
    ADD = mybir.AluOpType.add

    nc = bacc.Bacc(None, target_bir_lowering=False, debug=True)

    xqT = nc.dram_tensor("xqT", [D, S], F32R, kind="ExternalInput")
    xkT = nc.dram_tensor("xkT", [D, S], F32R, kind="ExternalInput")
    xvT = nc.dram_tensor("xvT", [D, S], F32R, kind="ExternalInput")
    wqT = nc.dram_tensor("wqT", [D, DG], F32R, kind="ExternalInput")
    wkT = nc.dram_tensor("wkT", [D, DG], F32R, kind="ExternalInput")
    wvT = nc.dram_tensor("wvT", [D, DG], F32R, kind="ExternalInput")
    woT = nc.dram_tensor("woT", [DG, D], F32R, kind="ExternalInput")
    bqc = nc.dram_tensor("bqc", [128, 4], F32, kind="ExternalInput")
    bkc = nc.dram_tensor("bkc", [128, 4], F32, kind="ExternalInput")
    bvr = nc.dram_tensor("bvr", [DG], F32, kind="ExternalInput")
    y_d = nc.dram_tensor("y", [S, D], F32R, kind="ExternalOutput")

    with (
        tile.TileContext(nc) as tc,
        nc.allow_low_precision(reason="float32r carries full fp32 bits"),
        tc.tile_pool(name="persist", bufs=1) as persist,
    ):
        QT = [persist.tile([128, S], F32R, name=f"QT{p}") for p in range(4)]
        KT = [persist.tile([128, S], F32R, name=f"KT{p}") for p in range(4)]
        AT = QT  # attention output overwrites QT columns already consumed
        vones = [persist.tile([128, 8, 65], BF16, name=f"vones{s}") for s in range(16)]
        bq_sb = persist.tile([128, 4], F32, name="bq_sb")
        bk_sb = persist.tile([128, 4], F32, name="bk_sb")
        bv_sb = persist.tile([128, DG], F32, name="bv_sb")

        nc.gpsimd.dma_start(bq_sb[:], bqc[:])
        nc.gpsimd.dma_start(bk_sb[:], bkc[:])
        bv_ap = bvr[:]
        bv_bcast = bass.AP(tensor=bv_ap.tensor, offset=bv_ap.offset, ap=[[0, 128], *bv_ap.ap])
        nc.gpsimd.dma_start(bv_sb[:], bv_bcast)
        for s in range(16):
            nc.vector.memset(vones[s][:, :, 64:65], 1.0)

        owp_cm = tc.tile_pool(name="out_w", bufs=1)
        owp = owp_cm.__enter__()

        with (
            tc.tile_pool(name="proj", bufs=2) as stage,
            tc.tile_pool(name="psproj", bufs=2, space="PSUM") as psp,
        ):
            # ---- V projection first (vones feeds every PV) ----
            kproj_cm = tc.tile_pool(name="kproj", bufs=2)
            kstage = kproj_cm.__enter__()
            wk = kstage.tile([128, 8, DG], F32R, name="wk", bufs=1)

            with tc.tile_pool(name="vproj", bufs=2) as vstage:
                wv = vstage.tile([128, 8, DG], F32R, name="wv", bufs=1)
                for i in range(8):
                    nc.gpsimd.dma_start(wv[:, i, :], wvT[i * 128 : (i + 1) * 128, :])
                # wk in parallel on the sync/scalar queues ahead of xv
                for i in range(8):
                    eng = nc.sync if i % 2 == 0 else nc.scalar
                    eng.dma_start(wk[:, i, :], wkT[i * 128 : (i + 1) * 128, :])
                for vc in range(8):
                    vs = slice(vc * 256, (vc + 1) * 256)
                    xv = vstage.tile([128, 8, 256], F32R, name="xv")
                    for i in range(8):
                        eng = (nc.sync, nc.scalar, nc.gpsimd)[i % 3]
                        eng.dma_start(xv[:, i, :], xvT[i * 128 : (i + 1) * 128, vs])
                    for sbi in range(2):
                        sb = vc * 2 + sbi
                        vp = psp.tile([128, DG], F32, name="pp")
                        for i in range(8):
                            nc.tensor.matmul(
                                vp[:],
                                xv[:, i, sbi * 128 : (sbi + 1) * 128],
                                wv[:, i, :],
                                start=(i == 0),
                                stop=(i == 7),
                            )
                        nc.vector.tensor_add(
                            vones[sb][:, :, 0:64],
                            vp[:].rearrange("p (h d) -> p h d", h=8),
                            bv_sb[:].rearrange("p (h d) -> p h d", h=8),
                        )

            # ---- K projection (all head groups) ----
            for qc in range(4):
                qs = slice(qc * 512, (qc + 1) * 512)
                xsk = kstage.tile([128, 8, 512], F32R, name="xsk")
                for i in range(8):
                    eng = (nc.sync, nc.scalar, nc.gpsimd)[i % 3]
                    eng.dma_start(xsk[:, i, :], xkT[i * 128 : (i + 1) * 128, qs])
                for p in range(4):
                    pp = psp.tile([128, 512], F32, name="pp")
                    for i in range(8):
                        nc.tensor.matmul(
                            pp[:],
                            wk[:, i, p * 128 : (p + 1) * 128],
                            xsk[:, i, :],
                            start=(i == 0),
                            stop=(i == 7),
                        )
                    nc.vector.tensor_scalar_add(
                        KT[p][:, qs], pp[:], bk_sb[:, p : p + 1]
                    )
            kproj_cm.__exit__(None, None, None)

            # Q weights + out-proj weights prefetch (on gpsimd)
            wq = stage.tile([128, 8, DG], F32R, name="wq", bufs=1)
            for i in range(8):
                nc.gpsimd.dma_start(wq[:, i, :], wqT[i * 128 : (i + 1) * 128, :])
            wo = owp.tile([128, 4, D], F32R, name="wo")
            for p_ in range(4):
                nc.gpsimd.dma_start(wo[:, p_, :], woT[p_ * 128 : (p_ + 1) * 128, :])

            # ---- attention, software-pipelined one iteration deep ----
            with (
                tc.tile_pool(name="attn_sb", bufs=8) as asb,
                tc.tile_pool(name="norm_sb", bufs=2) as nsb,
                tc.tile_pool(name="ps_st", bufs=2, space="PSUM") as ps_st,
                tc.tile_pool(name="ps_pv", bufs=1, space="PSUM") as ps_pv,
            ):
                def q_chunk_half(qc, half):
                    qs2 = slice(qc * 512 + half * 256, qc * 512 + half * 256 + 256)
                    xsq = stage.tile([128, 8, 256], F32R, name="xsq", bufs=1)
                    for i in range(8):
                        eng = nc.sync if i % 2 == 0 else nc.gpsimd
                        eng.dma_start(xsq[:, i, :], xqT[i * 128 : (i + 1) * 128, qs2])
                    for p in range(4):
                        pp = psp.tile([128, 512], F32, name="pp")
                        for i in range(8):
                            nc.tensor.matmul(
                                pp[:, 0:256],
                                wq[:, i, p * 128 : (p + 1) * 128],
                                xsq[:, i, :],
                                start=(i == 0),
                                stop=(i == 7),
                            )
                        nc.vector.tensor_scalar_add(
                            QT[p][:, qs2], pp[:, 0:256], bq_sb[:, p : p + 1]
                        )

                def sc_chunks(p, qs):
                    ech_tiles = []
                    for c in range(4):
                        ech = asb.tile([128, 2, 4, 512], BF16, name="ech")
                        ech_tiles.append(ech)
                        for kbi in range(4):
                            kb = c * 4 + kbi
                            st = ps_st.tile([128, 2, 512], F32, name="st")
                            for h in range(2):
                                nc.tensor.matmul(
                                    st[:, h, :],
                                    KT[p][h * 64 : (h + 1) * 64, kb * 128 : (kb + 1) * 128],
                                    QT[p][h * 64 : (h + 1) * 64, qs],
                                    start=True,
                                    stop=True,
                                )
                            if kb in DVE_KBS:
                                nc.vector.tensor_scalar(
                                    ech[:, :, kbi, :].bitcast(I16),
                                    st[:],
                                    EXP_A2,
                                    EXP_B2,
                                    MULT,
                                    ADD,
                                )
                            else:
                                nc.scalar.activation(
                                    out=ech[:, :, kbi, :], in_=st[:], func=EXP,
                                    scale=0.125,
                                )
                    return ech_tiles

                def pv_and_norm(pend):
                    pp_, pqs, petiles = pend
                    pv = [ps_pv.tile([128, 512], F32, name=f"pv{h}") for h in range(2)]
                    for half in range(2):
                        for h in range(2):
                            hidx = pp_ * 2 + h
                            for kbj in range(8):
                                c, kbi = divmod(kbj, 4)
                                nc.tensor.matmul(
                                    pv[h][0:65, :],
                                    vones[half * 8 + kbj][:, hidx, :],
                                    petiles[half * 2 + c][:, h, kbi, :],
                                    start=(half == 0 and kbj == 0),
                                    stop=(half == 1 and kbj == 7),
                                )
                    # normalize: 1/Z broadcast multiply, written into AT(=QT)
                    rec = []
                    for h in range(2):
                        z = nsb.tile([1, 512], F32, name=f"z{h}", bufs=1)
                        nc.vector.tensor_copy(z[:], pv[h][64:65, :])
                        r = nsb.tile([1, 512], F32, name=f"rec{h}", bufs=1)
                        nc.vector.reciprocal_approx_fast(out=r[:], in_=z[:])
                        rec.append(r)
                    for h in range(2):
                        rb = nsb.tile([64, 512], F32, name=f"rb{h}", bufs=1)
                        nc.gpsimd.partition_broadcast(rb[:], rec[h][:])
                        nc.vector.tensor_mul(
                            AT[pp_][h * 64 : (h + 1) * 64, pqs],
                            pv[h][0:64, :],
                            rb[:],
                        )

                pend = None
                for qc in range(4):
                    q_chunk_half(qc, 0)
                    q_chunk_half(qc, 1)
                    for p in range(4):
                        qs = slice(qc * 512, (qc + 1) * 512)
                        ech_tiles = sc_chunks(p, qs)
                        if pend is not None:
                            pv_and_norm(pend)
                        pend = (p, qs, ech_tiles)
                pv_and_norm(pend)

        # ---- output projection (partial y for this head-group) ----
        with (
            tc.tile_pool(name="out_y", bufs=3) as oy,
            tc.tile_pool(name="ps_y", bufs=4, space="PSUM") as ps_y,
        ):
            for sb in range(16):
                ss = slice(sb * 128, (sb + 1) * 128)
                ys = oy.tile([128, 2, 512], F32R, name="ys")
                for oc in range(2):
                    yp = ps_y.tile([128, 512], F32, name="yp")
                    for p in range(4):
                        nc.tensor.matmul(
                            yp[:],
                            AT[p][:, ss],
                            wo[:, p, oc * 512 : (oc + 1) * 512],
                            start=(p == 0),
                            stop=(p == 3),
                        )
                    if oc == 0:
                        nc.vector.tensor_copy(ys[:, oc, :], yp[:])
                    else:
                        nc.scalar.copy(ys[:, oc, :], yp[:])
                    eng = (nc.gpsimd, nc.sync, nc.scalar)[(2 * sb + oc) % 3]
                    eng.dma_start(
                        y_d[ss, oc * 512 : (oc + 1) * 512], ys[:, oc, :]
                    )
        owp_cm.__exit__(None, None, None)

    nc.compile()
    return nc


def _get_nc():
    if "nc" not in _NC_CACHE:
        _NC_CACHE["nc"] = _build_nc()
    return _NC_CACHE["nc"]


def kernel(**inputs):
    from concourse import bass_utils

    q, k, v = inputs["query"], inputs["key"], inputs["value"]
    Wq, Wk, Wv, Wo = inputs["Wq"], inputs["Wk"], inputs["Wv"], inputs["Wo"]
    bq, bk, bv, bo = inputs["bq"], inputs["bk"], inputs["bv"], inputs["bo"]

    nc = _get_nc()
    in_maps = []
    for c in range(NCORES):
        b, hg = divmod(c, 2)
        r0 = hg * DG
        rs = slice(r0, r0 + DG)
        in_maps.append(
            {
                "xqT": np.ascontiguousarray(q[b].T),
                "xkT": np.ascontiguousarray(k[b].T),
                "xvT": np.ascontiguousarray(v[b].T),
                "wqT": np.ascontiguousarray(Wq[rs, :].T),
                "wkT": np.ascontiguousarray(Wk[rs, :].T),
                "wvT": np.ascontiguousarray(Wv[rs, :].T),
                "woT": np.ascontiguousarray(Wo[:, rs].T),
                "bqc": np.ascontiguousarray(bq[rs].reshape(4, 128).T),
                "bkc": np.ascontiguousarray(bk[rs].reshape(4, 128).T),
                "bvr": np.ascontiguousarray(bv[rs]),
            }
        )
    import os

    trace = bool(os.environ.get("KERNEL_TRACE"))
    res = bass_utils.run_bass_kernel_spmd(
        nc, in_maps, core_ids=list(range(NCORES)), trace=trace
    )
    global LAST_EXEC_NS, LAST_RESULTS
    LAST_EXEC_NS = res.exec_time_ns
    LAST_RESULTS = res.results
    out = np.empty((B, S, D), np.float32)
    for b in range(B):
        out[b] = res.results[2 * b]["y"] + res.results[2 * b + 1]["y"] + bo[None, :]
    return out


# revision 35
# speedup vs baseline: 1.0577x; 1.0577x over previous
import numpy as np

# nn_MultiHeadedAttention: B=4, S=2048, D_MODEL=1024, H=16, D_K=64, fp32.
# Sharding: 8 cores = 4 batches x 2 head-groups (8 heads each).
# Per-core pipeline (PE-bound, matmuls fp32r at 1 cyc/row):
#   V-proj -> K-proj -> per qc: [Q-proj(qc) -> attention(qc, p=0..3)] -> out-proj
# Attention is software-pipelined one iteration deep: iteration j emits
# scores(j) then PV+normalize of j-1, so the exp of scores(j) has a full
# iteration of slack and the PE never waits on the activation engines.
# Scores ST[k,q] as row-tiled head pairs (fp32r, concurrent on HW), PV in
# bf16 with appended ones-column (softmax denominator free in row 64), exp
# split: exact on ACT (11/16 chunks), Schraudolph bit-trick int16->bf16 on
# DVE (5/16). Fast-approx reciprocal (after a partition-0 staging copy),
# normalize via gpsimd partition_broadcast + DVE multiply written into QT's
# consumed columns (attention output aliases QT). Host sums the two
# head-group partials and adds bo.

B, S, D, H, DK = 4, 2048, 1024, 16, 64
NCORES = 8
DG = 512  # dims per head-group (8 heads x 64)

# Schraudolph exp -> bf16 bits: int16(st*A2 + B2) ~= bf16(exp(st/8))
EXP_A2 = float(2**7 * 0.125 / np.log(2.0))
EXP_B2 = float(127 * 2**7 - 290000.0 / 65536.0)
DVE_KBS = (3, 7, 9, 11, 13, 15)  # kb chunks exp'd on Vector via bit-trick

_NC_CACHE = {}
LAST_EXEC_NS = None


def _build_nc():
    import concourse.bacc as bacc
    import concourse.bass as bass
    import concourse.tile as tile
    from concourse import mybir

    F32R = mybir.dt.float32r
    F32 = mybir.dt.float32
    BF16 = mybir.dt.bfloat16
    I16 = mybir.dt.int16
    EXP = mybir.ActivationFunctionType.Exp
    MULT = mybir.AluOpType.mult
    ADD = mybir.AluOpType.add

    nc = bacc.Bacc(None, target_bir_lowering=False, debug=True)

    xqT = nc.dram_tensor("xqT", [D, S], F32R, kind="ExternalInput")
    xkT = nc.dram_tensor("xkT", [D, S], F32R, kind="ExternalInput")
    xvT = nc.dram_tensor("xvT", [D, S], F32R, kind="ExternalInput")
    wqT = nc.dram_tensor("wqT", [D, DG], F32R, kind="ExternalInput")
    wkT = nc.dram_tensor("wkT", [D, DG], F32R, kind="ExternalInput")
    wvT = nc.dram_tensor("wvT", [D, DG], F32R, kind="ExternalInput")
    woT = nc.dram_tensor("woT", [DG, D], F32R, kind="ExternalInput")
    bqc = nc.dram_tensor("bqc", [128, 4], F32, kind="ExternalInput")
    bkc = nc.dram_tensor("bkc", [128, 4], F32, kind="ExternalInput")
    bvr = nc.dram_tensor("bvr", [DG], F32, kind="ExternalInput")
    y_d = nc.dram_tensor("y", [S, D], F32R, kind="ExternalOutput")

    with (
        tile.TileContext(nc) as tc,
        nc.allow_low_precision(reason="float32r carries full fp32 bits"),
        tc.tile_pool(name="persist", bufs=1) as persist,
    ):
        QT = [persist.tile([128, S], F32R, name=f"QT{p}") for p in range(4)]
        KT = [persist.tile([128, S], F32R, name=f"KT{p}") for p in range(4)]
        AT = QT  # attention output overwrites QT columns already consumed
        vones = [persist.tile([128, 8, 65], BF16, name=f"vones{s}") for s in range(16)]
        bq_sb = persist.tile([128, 4], F32, name="bq_sb")
        bk_sb = persist.tile([128, 4], F32, name="bk_sb")
        bv_sb = persist.tile([128, DG], F32, name="bv_sb")

        nc.gpsimd.dma_start(bq_sb[:], bqc[:])
        nc.gpsimd.dma_start(bk_sb[:], bkc[:])
        bv_ap = bvr[:]
        bv_bcast = bass.AP(tensor=bv_ap.tensor, offset=bv_ap.offset, ap=[[0, 128], *bv_ap.ap])
        nc.gpsimd.dma_start(bv_sb[:], bv_bcast)
        for s in range(16):
            nc.vector.memset(vones[s][:, :, 64:65], 1.0)

        owp_cm = tc.tile_pool(name="out_w", bufs=1)
        owp = owp_cm.__enter__()

        with (
            tc.tile_pool(name="proj", bufs=2) as stage,
            tc.tile_pool(name="psproj", bufs=2, space="PSUM") as psp,
        ):
            # ---- V projection first (vones feeds every PV) ----
            with tc.tile_pool(name="vproj", bufs=2) as vstage:
                wv = vstage.tile([128, 8, DG], F32R, name="wv", bufs=1)
                for i in range(8):
                    nc.gpsimd.dma_start(wv[:, i, :], wvT[i * 128 : (i + 1) * 128, :])
                for vc in range(8):
                    vs = slice(vc * 256, (vc + 1) * 256)
                    xv = vstage.tile([128, 8, 256], F32R, name="xv")
                    for i in range(8):
                        eng = (nc.sync, nc.scalar, nc.gpsimd)[i % 3]
                        eng.dma_start(xv[:, i, :], xvT[i * 128 : (i + 1) * 128, vs])
                    for sbi in range(2):
                        sb = vc * 2 + sbi
                        vp = psp.tile([128, DG], F32, name="pp")
                        for i in range(8):
                            nc.tensor.matmul(
                                vp[:],
                                xv[:, i, sbi * 128 : (sbi + 1) * 128],
                                wv[:, i, :],
                                start=(i == 0),
                                stop=(i == 7),
                            )
                        nc.vector.tensor_add(
                            vones[sb][:, :, 0:64],
                            vp[:].rearrange("p (h d) -> p h d", h=8),
                            bv_sb[:].rearrange("p (h d) -> p h d", h=8),
                        )

            # ---- K projection (all head groups) ----
            with tc.tile_pool(name="kproj", bufs=2) as kstage:
                wk = kstage.tile([128, 8, DG], F32R, name="wk", bufs=1)
                for i in range(8):
                    nc.gpsimd.dma_start(wk[:, i, :], wkT[i * 128 : (i + 1) * 128, :])
                for qc in range(4):
                    qs = slice(qc * 512, (qc + 1) * 512)
                    xsk = kstage.tile([128, 8, 512], F32R, name="xsk")
                    for i in range(8):
                        eng = nc.sync if i % 2 == 0 else nc.scalar
                        eng.dma_start(xsk[:, i, :], xkT[i * 128 : (i + 1) * 128, qs])
                    for p in range(4):
                        pp = psp.tile([128, 512], F32, name="pp")
                        for i in range(8):
                            nc.tensor.matmul(
                                pp[:],
                                wk[:, i, p * 128 : (p + 1) * 128],
                                xsk[:, i, :],
                                start=(i == 0),
                                stop=(i == 7),
                            )
                        nc.vector.tensor_scalar_add(
                            KT[p][:, qs], pp[:], bk_sb[:, p : p + 1]
                        )

            # Q weights + out-proj weights prefetch (on gpsimd)
            wq = stage.tile([128, 8, DG], F32R, name="wq", bufs=1)
            for i in range(8):
                nc.gpsimd.dma_start(wq[:, i, :], wqT[i * 128 : (i + 1) * 128, :])
            wo = owp.tile([128, 4, D], F32R, name="wo")
            for p_ in range(4):
                nc.gpsimd.dma_start(wo[:, p_, :], woT[p_ * 128 : (p_ + 1) * 128, :])

            # ---- attention, software-pipelined one iteration deep ----
            with (
                tc.tile_pool(name="attn_sb", bufs=8) as asb,
                tc.tile_pool(name="norm_sb", bufs=2) as nsb,
                tc.tile_pool(name="ps_st", bufs=2, space="PSUM") as ps_st,
                tc.tile_pool(name="ps_pv", bufs=1, space="PSUM") as ps_pv,
            ):
                def q_chunk_half(qc, half):
                    qs2 = slice(qc * 512 + half * 256, qc * 512 + half * 256 + 256)
                    xsq = stage.tile([128, 8, 256], F32R, name="xsq", bufs=2)
                    for i in range(8):
                        eng = nc.sync if i % 2 == 0 else nc.gpsimd
                        eng.dma_start(xsq[:, i, :], xqT[i * 128 : (i + 1) * 128, qs2])
                    for p in range(4):
                        pp = psp.tile([128, 512], F32, name="pp")
                        for i in range(8):
                            nc.tensor.matmul(
                                pp[:, 0:256],
                                wq[:, i, p * 128 : (p + 1) * 128],
                                xsq[:, i, :],
                                start=(i == 0),
                                stop=(i == 7),
                            )
                        nc.vector.tensor_scalar_add(
                            QT[p][:, qs2], pp[:, 0:256], bq_sb[:, p : p + 1]
                        )

                def sc_chunks(p, qs):
                    ech_tiles = []
                    for c in range(4):
                        ech = asb.tile([128, 2, 4, 512], BF16, name="ech")
                        ech_tiles.append(ech)
                        for kbi in range(4):
                            kb = c * 4 + kbi
                            st = ps_st.tile([128, 2, 512], F32, name="st")
                            for h in range(2):
                                nc.tensor.matmul(
                                    st[:, h, :],
                                    KT[p][h * 64 : (h + 1) * 64, kb * 128 : (kb + 1) * 128],
                                    QT[p][h * 64 : (h + 1) * 64, qs],
                                    start=True,
                                    stop=True,
                                )
                            if kb in DVE_KBS:
                                nc.vector.tensor_scalar(
                                    ech[:, :, kbi, :].bitcast(I16),
                                    st[:],
                                    EXP_A2,
                                    EXP_B2,
                                    MULT,
                                    ADD,
                                )
                            else:
                                nc.scalar.activation(
                                    out=ech[:, :, kbi, :], in_=st[:], func=EXP,
                                    scale=0.125,
                                )
                    return ech_tiles

                def pv_and_norm(pend):
                    pp_, pqs, petiles = pend
                    pv = [ps_pv.tile([128, 512], F32, name=f"pv{h}") for h in range(2)]
                    for half in range(2):
                        for h in range(2):
                            hidx = pp_ * 2 + h
                            for kbj in range(8):
                                c, kbi = divmod(kbj, 4)
                                nc.tensor.matmul(
                                    pv[h][0:65, :],
                                    vones[half * 8 + kbj][:, hidx, :],
                                    petiles[half * 2 + c][:, h, kbi, :],
                                    start=(half == 0 and kbj == 0),
                                    stop=(half == 1 and kbj == 7),
                                )
                    # normalize: 1/Z broadcast multiply, written into AT(=QT)
                    rec = []
                    for h in range(2):
                        z = nsb.tile([1, 512], F32, name=f"z{h}", bufs=1)
                        nc.vector.tensor_copy(z[:], pv[h][64:65, :])
                        r = nsb.tile([1, 512], F32, name=f"rec{h}", bufs=1)
                        nc.vector.reciprocal_approx_fast(out=r[:], in_=z[:])
                        rec.append(r)
                    for h in range(2):
                        rb = nsb.tile([64, 512], F32, name=f"rb{h}", bufs=1)
                        nc.gpsimd.partition_broadcast(rb[:], rec[h][:])
                        nc.vector.tensor_mul(
                            AT[pp_][h * 64 : (h + 1) * 64, pqs],
                            pv[h][0:64, :],
                            rb[:],
                        )

                pend = None
                for qc in range(4):
                    q_chunk_half(qc, 0)
                    q_chunk_half(qc, 1)
                    for p in range(4):
                        qs = slice(qc * 512, (qc + 1) * 512)
                        ech_tiles = sc_chunks(p, qs)
                        if pend is not None:
                            pv_and_norm(pend)
                        pend = (p, qs, ech_tiles)
                pv_and_norm(pend)

        # ---- output projection (partial y for this head-group) ----
        with (
            tc.tile_pool(name="out_y", bufs=3) as oy,
            tc.tile_pool(name="ps_y", bufs=4, space="PSUM") as ps_y,
        ):
            for sb in range(16):
                ss = slice(sb * 128, (sb + 1) * 128)
                ys = oy.tile([128, 2, 512], F32R, name="ys")
                for oc in range(2):
                    yp = ps_y.tile([128, 512], F32, name="yp")
                    for p in range(4):
                        nc.tensor.matmul(
                            yp[:],
                            AT[p][:, ss],
                            wo[:, p, oc * 512 : (oc + 1) * 512],
                            start=(p == 0),
                            stop=(p == 3),
                        )
                    if oc == 0:
                        nc.vector.tensor_copy(ys[:, oc, :], yp[:])
                    else:
                        nc.scalar.copy(ys[:, oc, :], yp[:])
                    eng = (nc.gpsimd, nc.sync, nc.scalar)[(2 * sb + oc) % 3]
                    eng.dma_start(
                        y_d[ss, oc * 512 : (oc + 1) * 512], ys[:, oc, :]
                    )
        owp_cm.__exit__(None, None, None)

    nc.compile()
    return nc


def _get_nc():
    if "nc" not in _NC_CACHE:
        _NC_CACHE["nc"] = _build_nc()
    return _NC_CACHE["nc"]


def kernel(**inputs):
    from concourse import bass_utils

    q, k, v = inputs["query"], inputs["key"], inputs["value"]
    Wq, Wk, Wv, Wo = inputs["Wq"], inputs["Wk"], inputs["Wv"], inputs["Wo"]
    bq, bk, bv, bo = inputs["bq"], inputs["bk"], inputs["bv"], inputs["bo"]

    nc = _get_nc()
    in_maps = []
    for c in range(NCORES):
        b, hg = divmod(c, 2)
        r0 = hg * DG
        rs = slice(r0, r0 + DG)
        in_maps.append(
            {
                "xqT": np.ascontiguousarray(q[b].T),
                "xkT": np.ascontiguousarray(k[b].T),
                "xvT": np.ascontiguousarray(v[b].T),
                "wqT": np.ascontiguousarray(Wq[rs, :].T),
                "wkT": np.ascontiguousarray(Wk[rs, :].T),
                "wvT": np.ascontiguousarray(Wv[rs, :].T),
                "woT": np.ascontiguousarray(Wo[:, rs].T),
                "bqc": np.ascontiguousarray(bq[rs].reshape(4, 128).T),
                "bkc": np.ascontiguousarray(bk[rs].reshape(4, 128).T),
                "bvr": np.ascontiguousarray(bv[rs]),
            }
        )
    import os

    trace = bool(os.environ.get("KERNEL_TRACE"))
    res = bass_utils.run_bass_kernel_spmd(
        nc, in_maps, core_ids=list(range(NCORES)), trace=trace
    )
    global LAST_EXEC_NS, LAST_RESULTS
    LAST_EXEC_NS = res.exec_time_ns
    LAST_RESULTS = res.results
    out = np.empty((B, S, D), np.float32)
    for b in range(B):
        out[b] = res.results[2 * b]["y"] + res.results[2 * b + 1]["y"] + bo[None, :]
    return out


# revision 38
# speedup vs baseline: 1.0694x; 1.0111x over previous
import numpy as np

# nn_MultiHeadedAttention: B=4, S=2048, D_MODEL=1024, H=16, D_K=64, fp32.
# Sharding: 8 cores = 4 batches x 2 head-groups (8 heads each).
# Per-core pipeline (PE-bound, matmuls fp32r at 1 cyc/row):
#   V-proj -> K-proj -> per qc: [Q-proj(qc) -> attention(qc, p=0..3)] -> out-proj
# Attention is software-pipelined one iteration deep: iteration j emits
# scores(j) then PV+normalize of j-1, so the exp of scores(j) has a full
# iteration of slack and the PE never waits on the activation engines.
# Scores ST[k,q] as row-tiled head pairs (fp32r, concurrent on HW), PV in
# bf16 with appended ones-column (softmax denominator free in row 64), exp
# split: exact on ACT (11/16 chunks), Schraudolph bit-trick int16->bf16 on
# DVE (5/16). Fast-approx reciprocal (after a partition-0 staging copy),
# normalize via gpsimd partition_broadcast + DVE multiply written into QT's
# consumed columns (attention output aliases QT). Host sums the two
# head-group partials and adds bo.

B, S, D, H, DK = 4, 2048, 1024, 16, 64
NCORES = 8
DG = 512  # dims per head-group (8 heads x 64)

# Schraudolph exp -> bf16 bits: int16(st*A2 + B2) ~= bf16(exp(st/8))
EXP_A2 = float(2**7 * 0.125 / np.log(2.0))
EXP_B2 = float(127 * 2**7 - 290000.0 / 65536.0)
DVE_KBS = (3, 7, 9, 11, 13, 15)  # kb chunks exp'd on Vector via bit-trick

_NC_CACHE = {}
LAST_EXEC_NS = None


def _build_nc():
    import concourse.bacc as bacc
    import concourse.bass as bass
    import concourse.tile as tile
    from concourse import mybir

    F32R = mybir.dt.float32r
    F32 = mybir.dt.float32
    BF16 = mybir.dt.bfloat16
    I16 = mybir.dt.int16
    EXP = mybir.ActivationFunctionType.Exp
    MULT = mybir.AluOpType.mult
    ADD = mybir.AluOpType.add

    nc = bacc.Bacc(None, target_bir_lowering=False, debug=True)

    xqT = nc.dram_tensor("xqT", [D, S], F32R, kind="ExternalInput")
    xkT = nc.dram_tensor("xkT", [D, S], F32R, kind="ExternalInput")
    xvT = nc.dram_tensor("xvT", [D, S], F32R, kind="ExternalInput")
    wqT = nc.dram_tensor("wqT", [D, DG], F32R, kind="ExternalInput")
    wkT = nc.dram_tensor("wkT", [D, DG], F32R, kind="ExternalInput")
    wvT = nc.dram_tensor("wvT", [D, DG], F32R, kind="ExternalInput")
    woT = nc.dram_tensor("woT", [DG, D], F32R, kind="ExternalInput")
    bqc = nc.dram_tensor("bqc", [128, 4], F32, kind="ExternalInput")
    bkc = nc.dram_tensor("bkc", [128, 4], F32, kind="ExternalInput")
    bvr = nc.dram_tensor("bvr", [DG], F32, kind="ExternalInput")
    y_d = nc.dram_tensor("y", [S, D], F32R, kind="ExternalOutput")

    with (
        tile.TileContext(nc) as tc,
        nc.allow_low_precision(reason="float32r carries full fp32 bits"),
        tc.tile_pool(name="persist", bufs=1) as persist,
    ):
        QT = [persist.tile([128, S], F32R, name=f"QT{p}") for p in range(4)]
        KT = [persist.tile([128, S], F32R, name=f"KT{p}") for p in range(4)]
        AT = QT  # attention output overwrites QT columns already consumed
        vones = [persist.tile([128, 8, 65], BF16, name=f"vones{s}") for s in range(16)]
        bq_sb = persist.tile([128, 4], F32, name="bq_sb")
        bk_sb = persist.tile([128, 4], F32, name="bk_sb")
        bv_sb = persist.tile([128, DG], F32, name="bv_sb")

        nc.gpsimd.dma_start(bq_sb[:], bqc[:])
        nc.gpsimd.dma_start(bk_sb[:], bkc[:])
        bv_ap = bvr[:]
        bv_bcast = bass.AP(tensor=bv_ap.tensor, offset=bv_ap.offset, ap=[[0, 128], *bv_ap.ap])
        nc.gpsimd.dma_start(bv_sb[:], bv_bcast)
        for s in range(16):
            nc.vector.memset(vones[s][:, :, 64:65], 1.0)

        owp_cm = tc.tile_pool(name="out_w", bufs=1)
        owp = owp_cm.__enter__()

        with (
            tc.tile_pool(name="proj", bufs=2) as stage,
            tc.tile_pool(name="psproj", bufs=2, space="PSUM") as psp,
        ):
            # ---- V projection first (vones feeds every PV) ----
            with tc.tile_pool(name="vproj", bufs=2) as vstage:
                wv = vstage.tile([128, 8, DG], F32R, name="wv", bufs=1)
                for i in range(8):
                    nc.gpsimd.dma_start(wv[:, i, :], wvT[i * 128 : (i + 1) * 128, :])
                for vc in range(8):
                    vs = slice(vc * 256, (vc + 1) * 256)
                    xv = vstage.tile([128, 8, 256], F32R, name="xv")
                    for i in range(8):
                        eng = (nc.sync, nc.scalar, nc.gpsimd)[i % 3]
                        eng.dma_start(xv[:, i, :], xvT[i * 128 : (i + 1) * 128, vs])
                    for sbi in range(2):
                        sb = vc * 2 + sbi
                        vp = psp.tile([128, DG], F32, name="pp")
                        for i in range(8):
                            nc.tensor.matmul(
                                vp[:],
                                xv[:, i, sbi * 128 : (sbi + 1) * 128],
                                wv[:, i, :],
                                start=(i == 0),
                                stop=(i == 7),
                            )
                        nc.vector.tensor_add(
                            vones[sb][:, :, 0:64],
                            vp[:].rearrange("p (h d) -> p h d", h=8),
                            bv_sb[:].rearrange("p (h d) -> p h d", h=8),
                        )

            # ---- K projection (all head groups) ----
            with tc.tile_pool(name="kproj", bufs=2) as kstage:
                wk = kstage.tile([128, 8, DG], F32R, name="wk", bufs=1)
                for i in range(8):
                    nc.gpsimd.dma_start(wk[:, i, :], wkT[i * 128 : (i + 1) * 128, :])
                for qc in range(4):
                    qs = slice(qc * 512, (qc + 1) * 512)
                    xsk = kstage.tile([128, 8, 512], F32R, name="xsk")
                    for i in range(8):
                        eng = nc.sync if i % 2 == 0 else nc.scalar
                        eng.dma_start(xsk[:, i, :], xkT[i * 128 : (i + 1) * 128, qs])
                    for p in range(4):
                        pp = psp.tile([128, 512], F32, name="pp")
                        for i in range(8):
                            nc.tensor.matmul(
                                pp[:],
                                wk[:, i, p * 128 : (p + 1) * 128],
                                xsk[:, i, :],
                                start=(i == 0),
                                stop=(i == 7),
                            )
                        nc.vector.tensor_scalar_add(
                            KT[p][:, qs], pp[:], bk_sb[:, p : p + 1]
                        )

            # Q weights + out-proj weights prefetch (on gpsimd)
            wq = stage.tile([128, 8, DG], F32R, name="wq", bufs=1)
            for i in range(8):
                nc.gpsimd.dma_start(wq[:, i, :], wqT[i * 128 : (i + 1) * 128, :])
            wo = owp.tile([128, 4, D], F32R, name="wo")
            for p_ in range(4):
                nc.gpsimd.dma_start(wo[:, p_, :], woT[p_ * 128 : (p_ + 1) * 128, :])

            # ---- attention, software-pipelined one iteration deep ----
            with (
                tc.tile_pool(name="attn_sb", bufs=7) as asb,
                tc.tile_pool(name="norm_sb", bufs=2) as nsb,
                tc.tile_pool(name="ps_st", bufs=2, space="PSUM") as ps_st,
                tc.tile_pool(name="ps_pv", bufs=1, space="PSUM") as ps_pv,
            ):
                def q_chunk_half(qc, half):
                    qs2 = slice(qc * 512 + half * 256, qc * 512 + half * 256 + 256)
                    xsq = stage.tile([128, 8, 256], F32R, name="xsq", bufs=2)
                    for i in range(8):
                        eng = nc.sync if i % 2 == 0 else nc.gpsimd
                        eng.dma_start(xsq[:, i, :], xqT[i * 128 : (i + 1) * 128, qs2])
                    for p in range(4):
                        pp = psp.tile([128, 512], F32, name="pp")
                        for i in range(8):
                            nc.tensor.matmul(
                                pp[:, 0:256],
                                wq[:, i, p * 128 : (p + 1) * 128],
                                xsq[:, i, :],
                                start=(i == 0),
                                stop=(i == 7),
                            )
                        nc.vector.tensor_scalar_add(
                            QT[p][:, qs2], pp[:, 0:256], bq_sb[:, p : p + 1]
                        )

                def sc_chunks(p, qs):
                    ech_tiles = []
                    for c in range(4):
                        ech = asb.tile([128, 2, 4, 512], BF16, name="ech")
                        ech_tiles.append(ech)
                        for kbi in range(4):
                            kb = c * 4 + kbi
                            st = ps_st.tile([128, 2, 512], F32, name="st")
                            for h in range(2):
                                nc.tensor.matmul(
                                    st[:, h, :],
                                    KT[p][h * 64 : (h + 1) * 64, kb * 128 : (kb + 1) * 128],
                                    QT[p][h * 64 : (h + 1) * 64, qs],
                                    start=True,
                                    stop=True,
                                )
                            if kb in DVE_KBS:
                                nc.vector.tensor_scalar(
                                    ech[:, :, kbi, :].bitcast(I16),
                                    st[:],
                                    EXP_A2,
                                    EXP_B2,
                                    MULT,
                                    ADD,
                                )
                            else:
                                nc.scalar.activation(
                                    out=ech[:, :, kbi, :], in_=st[:], func=EXP,
                                    scale=0.125,
                                )
                    return ech_tiles

                def pv_and_norm(pend):
                    pp_, pqs, petiles = pend
                    pv = [ps_pv.tile([128, 512], F32, name=f"pv{h}") for h in range(2)]
                    for half in range(2):
                        for h in range(2):
                            hidx = pp_ * 2 + h
                            for kbj in range(8):
                                c, kbi = divmod(kbj, 4)
                                nc.tensor.matmul(
                                    pv[h][0:65, :],
                                    vones[half * 8 + kbj][:, hidx, :],
                                    petiles[half * 2 + c][:, h, kbi, :],
                                    start=(half == 0 and kbj == 0),
                                    stop=(half == 1 and kbj == 7),
                                )
                    # normalize: 1/Z broadcast multiply, written into AT(=QT)
                    rec = []
                    for h in range(2):
                        z = nsb.tile([1, 512], F32, name=f"z{h}", bufs=1)
                        nc.vector.tensor_copy(z[:], pv[h][64:65, :])
                        r = nsb.tile([1, 512], F32, name=f"rec{h}", bufs=1)
                        nc.vector.reciprocal_approx_fast(out=r[:], in_=z[:])
                        rec.append(r)
                    for h in range(2):
                        rb = nsb.tile([64, 512], F32, name=f"rb{h}", bufs=1)
                        nc.gpsimd.partition_broadcast(rb[:], rec[h][:])
                        nc.vector.tensor_mul(
                            AT[pp_][h * 64 : (h + 1) * 64, pqs],
                            pv[h][0:64, :],
                            rb[:],
                        )

                def out_sb(sb):
                    # out-proj for one 128-token block whose AT columns are
                    # final; fills PE bubbles inside later attention iterations
                    ss = slice(sb * 128, (sb + 1) * 128)
                    for oc in range(2):
                        yp = psp.tile([128, 512], F32, name="pp")
                        for p_ in range(4):
                            nc.tensor.matmul(
                                yp[:],
                                AT[p_][:, ss],
                                wo[:, p_, oc * 512 : (oc + 1) * 512],
                                start=(p_ == 0),
                                stop=(p_ == 3),
                            )
                        yo = nsb.tile([128, 512], F32R, name="ysoc")
                        if oc == 0:
                            nc.vector.tensor_copy(yo[:], yp[:])
                        else:
                            nc.scalar.copy(yo[:], yp[:])
                        nc.sync.dma_start(y_d[ss, oc * 512 : (oc + 1) * 512], yo[:])

                pend = None
                for qc in range(4):
                    q_chunk_half(qc, 0)
                    q_chunk_half(qc, 1)
                    for p in range(4):
                        qs = slice(qc * 512, (qc + 1) * 512)
                        ech_tiles = sc_chunks(p, qs)
                        if qc >= 2:
                            out_sb((qc - 2) * 4 + p)
                        if pend is not None:
                            pv_and_norm(pend)
                        pend = (p, qs, ech_tiles)
                pv_and_norm(pend)

        # ---- output projection (remaining token blocks) ----
        with (
            tc.tile_pool(name="out_y", bufs=3) as oy,
            tc.tile_pool(name="ps_y", bufs=4, space="PSUM") as ps_y,
        ):
            for sb in range(8, 16):
                ss = slice(sb * 128, (sb + 1) * 128)
                ys = oy.tile([128, 2, 512], F32R, name="ys")
                for oc in range(2):
                    yp = ps_y.tile([128, 512], F32, name="yp")
                    for p in range(4):
                        nc.tensor.matmul(
                            yp[:],
                            AT[p][:, ss],
                            wo[:, p, oc * 512 : (oc + 1) * 512],
                            start=(p == 0),
                            stop=(p == 3),
                        )
                    if oc == 0:
                        nc.vector.tensor_copy(ys[:, oc, :], yp[:])
                    else:
                        nc.scalar.copy(ys[:, oc, :], yp[:])
                    eng = (nc.gpsimd, nc.sync, nc.scalar)[(2 * sb + oc) % 3]
                    eng.dma_start(
                        y_d[ss, oc * 512 : (oc + 1) * 512], ys[:, oc, :]
                    )
        owp_cm.__exit__(None, None, None)

    nc.compile()
    return nc


def _get_nc():
    if "nc" not in _NC_CACHE:
        _NC_CACHE["nc"] = _build_nc()
    return _NC_CACHE["nc"]


def kernel(**inputs):
    from concourse import bass_utils

    q, k, v = inputs["query"], inputs["key"], inputs["value"]
    Wq, Wk, Wv, Wo = inputs["Wq"], inputs["Wk"], inputs["Wv"], inputs["Wo"]
    bq, bk, bv, bo = inputs["bq"], inputs["bk"], inputs["bv"], inputs["bo"]

    nc = _get_nc()
    in_maps = []
    for c in range(NCORES):
        b, hg = divmod(c, 2)
        r0 = hg * DG
        rs = slice(r0, r0 + DG)
        in_maps.append(
            {
                "xqT": np.ascontiguousarray(q[b].T),
                "xkT": np.ascontiguousarray(k[b].T),
                "xvT": np.ascontiguousarray(v[b].T),
                "wqT": np.ascontiguousarray(Wq[rs, :].T),
                "wkT": np.ascontiguousarray(Wk[rs, :].T),
                "wvT": np.ascontiguousarray(Wv[rs, :].T),
                "woT": np.ascontiguousarray(Wo[:, rs].T),
                "bqc": np.ascontiguousarray(bq[rs].reshape(4, 128).T),
                "bkc": np.ascontiguousarray(bk[rs].reshape(4, 128).T),
                "bvr": np.ascontiguousarray(bv[rs]),
            }
        )
    import os

    trace = bool(os.environ.get("KERNEL_TRACE"))
    res = bass_utils.run_bass_kernel_spmd(
        nc, in_maps, core_ids=list(range(NCORES)), trace=trace
    )
    global LAST_EXEC_NS, LAST_RESULTS
    LAST_EXEC_NS = res.exec_time_ns
    LAST_RESULTS = res.results
    out = np.empty((B, S, D), np.float32)
    for b in range(B):
        out[b] = res.results[2 * b]["y"] + res.results[2 * b + 1]["y"] + bo[None, :]
    return out
